# revision 2
# baseline (speedup 1.0000x reference)
"""3-layer GCN (GCNConv x3) on 8 Trainium2 NeuronCores — v2.

Strategy (dst-sharded, matmul aggregation, chunk-pipelined AllGather):
  - out = dinv .* (A @ (dinv .* (x @ W))) + b  (norm factorized), A the
    self-loop-augmented adjacency.
  - Nodes sharded by contiguous range (12500/core). h~ rows stored PACKED
    (64 x dtype bytes, contiguous).
  - The per-layer AllGather is split into `nag` rank-sliced chunks over a
    PERMUTED node layout: chunk k carries slice k of every rank's shard, so
    the gathered buffer hg_k = concat_r h_loc_r[slice k] is contiguous and
    becomes gather WINDOW k (<= 32768 rows, int16 gather indices).
  - Each window is expanded (128B rows -> 256B-stride rows for bf16) by one
    strided DMA as soon as its AllGather lands; fp32 rows are already 256B
    so hg_k is gathered directly.
  - Aggregation is WINDOW-MAJOR: edges sorted by (window, dst block); psum
    pair-tiles are transient per (pair, window) and flushed into a persistent
    fp32 SBUF accumulator [64, nloc]. Gathers of window k overlap the
    AllGather of chunk k+1 (and the expand of k+1).
  - Epilogue (dinv mult, +bias/ReLU, feature-major store) fires per pair
    right after its last-window flush, so the next layer's chunked GEMM and
    AllGather start while this layer's tail is still aggregating.

kernel(**inputs) is self-contained: host-side numpy planning, Bass build,
compile+run via run_bass_kernel_spmd on cores 0-7, gather + transpose out.
"""

import numpy as np

P = 128


def _cfg_full(dtype="bfloat16"):
    return dict(
        n_nodes=100000,
        n_cores=8,
        d_in=128,
        d_hid=64,
        blk=128,  # dst-block size (one-hot width)
        nag=4,  # legacy window-count hint (nag_coll drives collectives)
        nag_coll=4,  # AllGather chunks per layer (pipelined with gathers)
        nsq=4,  # SWDGE queues: gather calls round-robin over 4 DMA queues
        batch=8192,  # max gather-call size in indices (multiple of 128;
        # 16384 overflows the SWDGE descriptor ring and wedges the device)
        dtype=dtype,  # "float32" or "bfloat16" for h~ / gather / matmul
        scratch=32768,  # SWDGE descriptor carveout bytes
    )


def _np_dt(dtype):
    if dtype == "float32":
        return np.float32
    import ml_dtypes

    return ml_dtypes.bfloat16


# ----------------------------------------------------------------------------
# Host planning
# ----------------------------------------------------------------------------


def _host_plan(x, edge_index, cfg):
    """Numpy preprocessing: norm factorization, permuted chunk layout, edge
    sorting/padding (window-major), the combined [gather-idx | dst-sel] int16
    side-array, and the (SPMD-uniform) emission schedule."""
    N = cfg["n_nodes"]
    NCORES = cfg["n_cores"]
    BLK = cfg["blk"]
    NAG = cfg["nag"]
    nloc = N // NCORES
    ntb = -(-nloc // BLK)  # dst blocks per core
    ndt = _np_dt(cfg["dtype"])

    # C = collectives per layer. Each chunk k carries slice k of every rank's
    # shard (rank-major in the gathered buffer), and splits into rank-group
    # WINDOWS of <= 32768 rows (int16 gather indices).
    C = int(cfg.get("nag_coll", 1 if cfg.get("one_ag") else NAG))
    base = -(-(-(-nloc // C)) // 256) * 256
    cuts = [min(i * base, nloc) for i in range(C + 1)]
    cuts = sorted(set(cuts))  # drop empty chunks (tiny nloc)
    sizes = np.diff(np.asarray(cuts)).astype(np.int64)
    C = len(sizes)
    windows = []  # (chunk, rank0, nranks, row0_in_chunk, wrow)
    for k in range(C):
        cs = int(sizes[k])
        rg = max(1, 32768 // cs)  # ranks per window
        for g in range(-(-NCORES // rg)):
            nr = min(rg, NCORES - g * rg)
            windows.append((k, g * rg, nr, g * rg * cs, nr * cs))
    nag = len(windows)
    wrows = [w[4] for w in windows]
    rg_of_chunk = [max(1, 32768 // int(sizes[k])) for k in range(C)]
    base_w = {}
    for i, w in enumerate(windows):
        base_w.setdefault(w[0], i)
    assert max(wrows) <= 32768

    e0 = np.asarray(edge_index[0], dtype=np.int64)
    e1 = np.asarray(edge_index[1], dtype=np.int64)
    loop = np.arange(N, dtype=np.int64)
    src = np.concatenate([e0, loop])
    dst = np.concatenate([e1, loop])

    deg = np.bincount(dst, minlength=N).astype(np.float64)
    dinv = np.where(deg > 0, 1.0 / np.sqrt(deg), 0.0).astype(np.float32)

    core = dst // nloc
    dloc = dst - core * nloc
    dblock = dloc // BLK
    dsel = (dloc - dblock * BLK).astype(np.int16)

    # window mapping of src: rank r, local row l -> chunk k, rank group g
    sr = src // nloc
    sl = src - sr * nloc
    cuts_a = np.asarray(cuts, dtype=np.int64)
    k_of = np.searchsorted(cuts_a, sl, side="right") - 1
    rg_a = np.asarray(rg_of_chunk, dtype=np.int64)
    base_w_a = np.asarray([base_w[k] for k in range(C)], dtype=np.int64)
    g_of = sr // rg_a[k_of]
    s_of = base_w_a[k_of] + g_of
    gsrc = (
        (sr - g_of * rg_a[k_of]) * sizes[k_of] + (sl - cuts_a[k_of])
    ).astype(np.int16)

    # counts per (core, dblock, window); pad to max over cores, mult of 128
    key_cbs = (core * ntb + dblock) * nag + s_of
    cnt = np.bincount(key_cbs, minlength=NCORES * ntb * nag).reshape(
        NCORES, ntb, nag
    )
    padded = ((cnt.max(axis=0) + P - 1) // P) * P  # [ntb, nag]

    # stream order: (core, window, dblock) — window-major; within a group,
    # ascending source row (HBM row-buffer locality for the gather)
    skey = (core * nag + s_of) * ntb + dblock
    if cfg.get("no_sort_idx"):
        order = np.argsort(skey, kind="stable")
    else:
        order = np.lexsort((gsrc.astype(np.int64), skey))
    g_sorted = gsrc[order]
    dsel_sorted = dsel[order]
    skey_cnt = np.bincount(skey, minlength=NCORES * nag * ntb)
    skey_off = np.zeros(len(skey_cnt) + 1, dtype=np.int64)
    np.cumsum(skey_cnt, out=skey_off[1:])

    SL = int(padded.sum())  # uniform per-core stream length
    gidx_streams = np.zeros((NCORES, SL), np.int16)
    dsel_streams = np.full((NCORES, SL), -1, np.int16)

    pos_of = {}
    pos = 0
    for s in range(nag):
        for b in range(ntb):
            pos_of[(b, s)] = pos
            pos += int(padded[b, s])
    assert pos == SL

    for c in range(NCORES):
        for s in range(nag):
            for b in range(ntb):
                k = (c * nag + s) * ntb + b
                i0, i1 = int(skey_off[k]), int(skey_off[k + 1])
                n = i1 - i0
                q = pos_of[(b, s)]
                gidx_streams[c, q : q + n] = g_sorted[i0:i1]
                dsel_streams[c, q : q + n] = dsel_sorted[i0:i1]

    if not cfg.get("no_stripe"):
        # The gather ucode assigns stream position p to SDMA engine p%16.
        # Counter-stripe each (block, window) group so engine e drains a
        # CONTIGUOUS ascending-source run (HBM locality), instead of every
        # 16th element of the sorted order.
        perm = np.empty(SL, np.int64)
        for (b, s), q0 in pos_of.items():
            m = int(padded[b, s])
            p = np.arange(m)
            perm[q0 : q0 + m] = q0 + (p % 16) * (m // 16) + p // 16
        gidx_streams = gidx_streams[:, perm]
        dsel_streams = dsel_streams[:, perm]

    # Emission schedule (uniform across cores): per window a list of gather
    # calls (n_idx, [dblock per chunk]).
    B = cfg["batch"]
    sched = []
    for s in range(nag):
        chunk_blocks = []
        for b in range(ntb):
            chunk_blocks += [b] * (int(padded[b, s]) // P)
        calls = []
        i = 0
        while i < len(chunk_blocks):
            take = min(B // P, len(chunk_blocks) - i)
            calls.append((take * P, chunk_blocks[i : i + take]))
            i += take
        sched.append(calls)

    # per (block, window) chunk counts for psum start/stop bookkeeping
    chunks_bs = {(b, s): int(padded[b, s]) // P for b in range(ntb) for s in range(nag)}

    # Combined meta array: per call [wrap16(gidx) | cols128(dsel)] int16.
    def wrap16(a):
        w = a.reshape(-1, 16).T
        return np.tile(w, (8, 1))

    def cols128(a):
        return a.reshape(-1, P).T

    metas = []
    for c in range(NCORES):
        parts = []
        q = 0
        for calls in sched:
            for n_idx, chunk_blocks in calls:
                gi = gidx_streams[c, q : q + n_idx]
                ds = dsel_streams[c, q : q + n_idx]
                parts.append(wrap16(gi))
                parts.append(cols128(ds))
                q += n_idx
        assert q == SL
        metas.append(np.ascontiguousarray(np.concatenate(parts, axis=1)))
    TCM = metas[0].shape[1]

    per_core = []
    for c in range(NCORES):
        nt128 = -(-nloc // P)
        dvp = np.zeros(nt128 * P, np.float32)
        dvp[:nloc] = dinv[c * nloc : (c + 1) * nloc]
        dinv_cols = np.ascontiguousarray(dvp.reshape(nt128, P).T)  # [128, nt128]
        dinv_rep = np.tile(dvp.reshape(1, nt128 * P), (64, 1)).astype(np.float32)
        xt = np.ascontiguousarray(
            np.asarray(x[c * nloc : (c + 1) * nloc], dtype=np.float32).T
        ).astype(ndt)
        per_core.append(
            dict(
                xt=xt,
                dinv_cols=dinv_cols,
                dinv_rep=np.ascontiguousarray(dinv_rep),
                meta=metas[c],
            )
        )

    iota = np.tile(np.arange(BLK, dtype=np.int16), (P, 1))

    plan = dict(
        nloc=nloc,
        ntb=ntb,
        nag=nag,
        windows=windows,
        cuts=cuts,
        sizes=[int(s) for s in sizes],
        wrows=wrows,
        SL=SL,
        TCM=TCM,
        sched=sched,
        chunks_bs=chunks_bs,
        per_core=per_core,
        iota=iota,
    )
    return plan


# ----------------------------------------------------------------------------
# Device program
# ----------------------------------------------------------------------------


def _emit_gather(
    nc, out_ap, in_ap, idxs_ap, num_idxs, elem_size, elem_step, queue_num=0
):
    """nc.gpsimd.dma_gather, or a direct emission when elem_size_bytes is not
    a multiple of 256 (that assert is a transpose-path restriction; the
    non-transpose HBM ucode only needs the row *stride* to be 256B-aligned)."""
    import concourse.mybir as mybir

    dt_size = mybir.dt.size(in_ap.dtype)
    if (elem_size * dt_size) % 256 == 0 and elem_step == elem_size:
        return nc.gpsimd.dma_gather(
            out_ap,
            in_ap,
            idxs_ap,
            num_idxs,
            num_idxs,
            elem_size,
            # single_packet coalesces each engine's whole descriptor stream
            # into one DMA packet; beyond ~64 descriptors that is out of spec
            # and hard-crashes the SDMA engine (NRT_EXEC_UNIT_UNRECOVERABLE).
            single_packet=False,
            queue_num=queue_num,
        )
    g = nc.gpsimd
    stride_bytes = elem_step * dt_size
    assert stride_bytes % 256 == 0
    _in_ap = g.lower_ap_dma(in_ap, for_custom_bir_dma=True)
    _idxs_ap = g.lower_ap(idxs_ap)
    _out_ap = g.lower_ap(out_ap)
    return g.add_instruction(
        mybir.InstDMAGatherAnt(
            name=g.bass.get_next_instruction_name(),
            ins=[*_in_ap, _idxs_ap, g.lower_val_access(g.to_reg(num_idxs))],
            outs=[_out_ap],
            transpose=False,
            num_idxs=num_idxs,
            elem_size=elem_size,
            stride_bytes_256=stride_bytes // 256,
            gen_mode=0,
            single_packet=False,
            queue_num=queue_num,
            sbuf_tokens_per_rank=0,
            sbuf_free_dim_per_rank=0,
            sbuf_free_dim_pad_per_rank=0,
            sbuf_byte_offset=0,
        )
    )


def _build_program(plan, cfg, b_nonzero, use_collective=True):
    import concourse.bacc as bacc
    import concourse.mybir as mybir
    import concourse.tile as tile

    dt = mybir.dt
    DT = dt.float32 if cfg["dtype"] == "float32" else dt.bfloat16
    F32 = dt.float32
    DIN, DH = cfg["d_in"], cfg["d_hid"]
    N = cfg["n_nodes"]
    BLK = cfg["blk"]
    NCORES = cfg["n_cores"]
    nloc, ntb, nag = plan["nloc"], plan["ntb"], plan["nag"]
    cuts, sizes, wrows = plan["cuts"], plan["sizes"], plan["wrows"]
    windows = plan["windows"]
    n_chunks = len(sizes)  # GEMM/AllGather chunks
    SL, TCM = plan["SL"], plan["TCM"]
    B = cfg["batch"]
    PB = 2 if BLK == 64 else 1  # blocks per psum pair-tile
    nt128 = -(-nloc // P)
    need_expand = DT != F32  # fp32 rows are already 256B
    row_elems = 256 // dt.size(DT)  # gather-source row stride (elements)

    nc = bacc.Bacc(
        None,
        target_bir_lowering=False,
        num_devices=NCORES,
        dynamic_dma_scratch_size=cfg["scratch"],
        num_swdge_queues=int(cfg.get("nsq", 1)),
    )

    xt_in = nc.dram_tensor("xt", [DIN, nloc], DT, kind="ExternalInput")
    dinv_cols = nc.dram_tensor(
        "dinv_cols", [P, nt128], F32, kind="ExternalInput"
    )
    dinv_rep = nc.dram_tensor(
        "dinv_rep", [64, nt128 * P], F32, kind="ExternalInput"
    )
    ws_in = [
        nc.dram_tensor("w1", [DIN, DH], DT, kind="ExternalInput"),
        nc.dram_tensor("w2", [DH, DH], DT, kind="ExternalInput"),
        nc.dram_tensor("w3", [DH, DH], DT, kind="ExternalInput"),
    ]
    bs_in = nc.dram_tensor("bs", [64, 3], F32, kind="ExternalInput")
    meta_in = nc.dram_tensor("meta", [P, TCM], dt.int16, kind="ExternalInput")
    iota_in = nc.dram_tensor("iota", [P, BLK], dt.int16, kind="ExternalInput")
    out_dram = nc.dram_tensor("out", [DH, nloc], F32, kind="ExternalOutput")

    h_loc = [
        nc.dram_tensor(f"h_loc{k}", [sizes[k], DH], DT) for k in range(n_chunks)
    ]
    hg = [
        nc.dram_tensor(f"hg{k}", [NCORES * sizes[k], DH], DT, addr_space="Shared")
        for k in range(n_chunks)
    ]
    if need_expand:
        h_w = [
            nc.dram_tensor(f"h_w{s}", [wrows[s], row_elems], DT)
            for s in range(nag)
        ]
    else:
        h_w = None  # fp32: windows gather straight from hg slices
    xt2 = nc.dram_tensor("xt2", [DH, nloc], DT)
    xt3 = nc.dram_tensor("xt3", [DH, nloc], DT)

    rg = [list(range(NCORES))]
    MCOLS = B // 16 + B // P  # meta tile columns per call (max)

    if cfg.get("null_kernel"):
        # same I/O signature, ~no work: for calibrating dispatch overhead
        with tile.TileContext(nc) as tc:
            with tc.tile_pool(name="p", bufs=1) as pool:
                z = pool.tile([64, P], F32)
                nc.vector.memset(z[:, :], 0.0)
                nc.sync.dma_start(out=out_dram[:, :P], in_=z[:, :])
        nc.compile()
        return nc

    with tile.TileContext(nc) as tc:
        with (
            tc.tile_pool(name="const", bufs=1) as cpool,
            tc.tile_pool(name="work", bufs=3) as wpool,
            tc.tile_pool(name="gath", bufs=2) as gpool,
            tc.tile_pool(name="onehot", bufs=2) as mpool,
            tc.tile_pool(name="idx", bufs=2) as ipool,
            tc.tile_pool(name="ps", bufs=2, space="PSUM") as pspool,
            tc.tile_pool(name="aggps", bufs=4, space="PSUM") as apool,
        ):
            w_sb = []
            for li, w in enumerate(ws_in):
                t = cpool.tile([w.shape[0], DH], DT, tag=f"w{li}")
                nc.sync.dma_start(out=t[:, :], in_=w[:, :])
                w_sb.append(t)
            b_sb = cpool.tile([64, 3], F32, tag="bs")
            nc.sync.dma_start(out=b_sb[:, :], in_=bs_in[:, :])
            dinvc_sb = cpool.tile([P, nt128], F32, tag="dinvc")
            nc.sync.dma_start(out=dinvc_sb[:, :], in_=dinv_cols[:, :])
            dinvr_sb = cpool.tile([64, nt128 * P], F32, tag="dinvr")
            nc.sync.dma_start(out=dinvr_sb[:, :], in_=dinv_rep[:, :])
            iota_sb = cpool.tile([P, BLK], dt.int16, tag="iota")
            nc.sync.dma_start(out=iota_sb[:, :], in_=iota_in[:, :])
            acc = cpool.tile([64, ntb * BLK], F32, tag="acc")

            xt_srcs = [xt_in, xt2, xt3]
            xt_dsts = [xt2, xt3, None]
            gq = [0]  # gather-call counter for queue round-robin

            for _rep in range(int(cfg.get("reps", 1))):
              for L in range(3):
                dk = DIN if L == 0 else DH
                xt_src = xt_srcs[L]
                # ---- phase 1: h~ = dinv .* (x @ W) per chunk, then AllGather
                for k in range(n_chunks):
                    r0k = cuts[k]
                    ntiles = -(-sizes[k] // P)
                    for tp in range(-(-ntiles // 2)):
                        t0 = 2 * tp
                        nt_in_pair = min(2, ntiles - t0)
                        r0 = t0 * P  # within chunk
                        rows = min(2 * P, sizes[k] - r0)
                        xts = wpool.tile([dk, 2 * P], DT, tag="xts")
                        nc.sync.dma_start(
                            out=xts[:, :rows],
                            in_=xt_src[:, r0k + r0 : r0k + r0 + rows],
                        )
                        hs = wpool.tile([P, 2, DH], DT, tag="hs")
                        for j in range(nt_in_pair):
                            t = t0 + j
                            rt = min(P, sizes[k] - t * P)
                            tg = (r0k + t * P) // P  # global 128-tile index
                            hp = pspool.tile([P, DH], F32, tag="hp")
                            nc.tensor.matmul(
                                hp[:rt, :],
                                lhsT=xts[:, j * P : j * P + rt],
                                rhs=w_sb[L][:, :],
                                start=True,
                                stop=True,
                            )
                            nc.scalar.activation(
                                hs[:rt, j, :],
                                hp[:rt, :],
                                mybir.ActivationFunctionType.Copy,
                                scale=dinvc_sb[:rt, tg : tg + 1],
                            )
                        if rows == 2 * P:
                            nc.sync.dma_start(
                                out=h_loc[k][r0 : r0 + rows, :].rearrange(
                                    "(c p) f -> p c f", p=P
                                ),
                                in_=hs[:, :, :],
                            )
                        else:
                            for j in range(nt_in_pair):
                                t = t0 + j
                                rt = min(P, sizes[k] - t * P)
                                nc.sync.dma_start(
                                    out=h_loc[k][t * P : t * P + rt, :],
                                    in_=hs[:rt, j, :],
                                )
                    # ---- AllGather chunk k
                    if cfg.get("skip_coll"):
                        pass
                    elif use_collective:
                        nc.gpsimd.collective_compute(
                            "AllGather",
                            mybir.AluOpType.bypass,
                            replica_groups=rg,
                            ins=[h_loc[k][:, :]],
                            outs=[hg[k][:, :]],
                        )
                    else:
                        nc.sync.dma_start(
                            out=hg[k][: sizes[k], :], in_=h_loc[k][:, :]
                        )
                # ---- phase 2: window-major aggregation
                if cfg.get("skip_agg"):
                    continue
                nc.vector.memset(acc[:, :], 0.0)
                mcol = 0
                seen = dict.fromkeys(range(ntb), 0)
                done = dict.fromkeys(range(ntb), 0)  # windows finished per blk
                total_bs = plan["chunks_bs"]
                epilogued = set()
                pair_tiles = {}

                def flush_pair(pr):
                    pt = pair_tiles.pop(pr)
                    c0 = pr * PB * BLK
                    rt = min(PB * BLK, ntb * BLK - c0)
                    nc.vector.tensor_tensor(
                        out=acc[:, c0 : c0 + rt],
                        in0=pt[:, :rt],
                        in1=acc[:, c0 : c0 + rt],
                        op=mybir.AluOpType.add,
                    )

                def epilogue_pair(pr):
                        c0 = pr * PB * BLK
                        rt = min(PB * BLK, nloc - c0)
                        u = wpool.tile([64, P], F32, tag="u")
                        nc.vector.tensor_tensor(
                            out=u[:, :rt],
                            in0=acc[:, c0 : c0 + rt],
                            in1=dinvr_sb[:, c0 : c0 + rt],
                            op=mybir.AluOpType.mult,
                        )
                        if L < 2:
                            us = wpool.tile([64, P], DT, tag="us")
                            nc.scalar.activation(
                                us[:, :rt],
                                u[:, :rt],
                                mybir.ActivationFunctionType.Relu,
                                bias=b_sb[:, L : L + 1] if b_nonzero else 0.0,
                            )
                            nc.sync.dma_start(
                                out=xt_dsts[L][:, c0 : c0 + rt],
                                in_=us[:, :rt],
                            )
                        else:
                            if b_nonzero:
                                nc.vector.tensor_scalar(
                                    u[:, :rt],
                                    u[:, :rt],
                                    b_sb[:, L : L + 1],
                                    None,
                                    mybir.AluOpType.add,
                                )
                            nc.sync.dma_start(
                                out=out_dram[:, c0 : c0 + rt], in_=u[:, :rt]
                            )

                for s in range(nag):
                    wk, _, _, wr0, wrow = windows[s]
                    # expand window s (bf16): 128B rows -> 256B-stride rows
                    if need_expand and not cfg.get("skip_coll"):
                        nc.scalar.dma_start(
                            out=h_w[s][:, :DH],
                            in_=hg[wk][wr0 : wr0 + wrow, :],
                        )
                    gather_src = (
                        h_w[s][:wrow, :DH]
                        if h_w is not None
                        else hg[wk][wr0 : wr0 + wrow, :DH]
                    )
                    for b in range(ntb):
                        if total_bs[(b, s)] == 0:
                            done[b] += 1

                    for n_idx, chunk_blocks in sched_calls(plan, s):
                        nch = n_idx // P
                        icols = n_idx // 16
                        mt = ipool.tile([P, MCOLS], dt.int16, tag="mt")
                        nc.sync.dma_start(
                            out=mt[:, : icols + nch],
                            in_=meta_in[:, mcol : mcol + icols + nch],
                        )
                        gt = gpool.tile([P, B // P, DH], DT, tag="gt")
                        if not cfg.get("skip_gather"):
                            _emit_gather(
                                nc,
                                gt[:, :nch, :],
                                gather_src,
                                mt[:, :icols],
                                n_idx,
                                DH,
                                row_elems,
                                queue_num=gq[0] % int(cfg.get("nsq", 1)),
                            )
                            gq[0] += 1
                        M = mpool.tile([P, B // P, BLK], DT, tag="M")
                        if not cfg.get("skip_onehot"):
                            nc.vector.tensor_tensor(
                                out=M[:, :nch, :],
                                in0=iota_sb[:]
                                .rearrange("p (c f) -> p c f", c=1)
                                .to_broadcast([P, nch, BLK]),
                                in1=mt[:, icols : icols + nch]
                                .rearrange("p (c f) -> p c f", f=1)
                                .to_broadcast([P, nch, BLK]),
                                op=mybir.AluOpType.is_equal,
                            )
                        for ci, b in enumerate(chunk_blocks):
                            pr = b // PB
                            if pr not in pair_tiles:
                                pt = apool.tile(
                                    [64, PB * BLK], F32, tag="aggps"
                                )
                                nc.vector.memset(pt[:, :], 0.0)
                                pair_tiles[pr] = pt
                            if not cfg.get("skip_matmul"):
                                rt = min(BLK, nloc - b * BLK)
                                half = (b % PB) * BLK
                                lhsT = (
                                    M[:, ci, :]
                                    if cfg.get("skip_gather")
                                    else gt[:, ci, :]
                                )
                                nc.tensor.matmul(
                                    pair_tiles[pr][:, half : half + rt],
                                    lhsT=lhsT,
                                    rhs=M[:, ci, :rt],
                                    start=False,
                                    stop=(seen[b] == total_bs[(b, s)] - 1),
                                    skip_group_check=True,
                                )
                            seen[b] += 1
                            if seen[b] == total_bs[(b, s)]:
                                seen[b] = 0
                                done[b] += 1
                                sib = (b ^ 1) if PB == 2 else ntb
                                if sib >= ntb or done[sib] >= done[b]:
                                    # both halves of the pair finished window s
                                    flush_pair(pr)
                                    if s == nag - 1:
                                        epilogue_pair(pr)
                                        epilogued.add(pr)
                        mcol += icols + nch
                    assert not pair_tiles, (L, s, list(pair_tiles))
                # pairs with no chunks in the last window still need output
                for pr in range(-(-ntb // PB)):
                    if pr not in epilogued:
                        epilogue_pair(pr)
                assert mcol == TCM
    nc.compile()
    return nc


def sched_calls(plan, s):
    return plan["sched"][s]


# ----------------------------------------------------------------------------
# Entry points
# ----------------------------------------------------------------------------


def build_and_run(inputs, cfg, trace=False):
    from concourse.bass_utils import run_bass_kernel_spmd

    x = np.asarray(inputs["x"])
    plan = _host_plan(x, np.asarray(inputs["edge_index"]), cfg)
    ndt = _np_dt(cfg["dtype"])

    bvals = [np.asarray(inputs[k], dtype=np.float32) for k in ("b1", "b2", "b3")]
    b_nonzero = any(np.any(b != 0) for b in bvals)
    bs = np.zeros((64, 3), np.float32)
    for i, b in enumerate(bvals):
        bs[: b.shape[0], i] = b

    nc = _build_program(plan, cfg, b_nonzero)

    ws = [
        np.ascontiguousarray(np.asarray(inputs[k], dtype=np.float32)).astype(ndt)
        for k in ("W1", "W2", "W3")
    ]
    in_maps = []
    for c in range(cfg["n_cores"]):
        pc = plan["per_core"][c]
        in_maps.append(
            {
                "xt": pc["xt"],
                "dinv_cols": pc["dinv_cols"],
                "dinv_rep": pc["dinv_rep"],
                "w1": ws[0],
                "w2": ws[1],
                "w3": ws[2],
                "bs": bs,
                "meta": pc["meta"],
                "iota": plan["iota"],
            }
        )

    res = run_bass_kernel_spmd(
        nc, in_maps, core_ids=list(range(cfg["n_cores"])), trace=trace
    )
    out = np.concatenate(
        [np.asarray(r["out"]).T for r in res.results], axis=0
    ).astype(np.float32)
    return out, res


def kernel(**inputs):
    cfg = _cfg_full(dtype="bfloat16")
    out, _ = build_and_run(inputs, cfg)
    return out
